# revision 9
# baseline (speedup 1.0000x reference)
"""VQ codebook encoder (LayerNorm -> nearest-codebook-entry -> straight-through
output + commitment loss) on 8 Trainium2 NeuronCores, data-parallel over batch.

Numerics: distances are computed as ns = q.e - ||e||^2/2 (argmax-equivalent to
argmin of squared distance; the ||q||^2 term is constant per token). The q.e
matmul runs as an fp16 hi/lo split (3 passes accumulated in one fp32 PSUM
group): q.e = qh.eh + (qh/64).(64*el) + ql.eh, which carries ~22 mantissa bits
per operand — argmin-exact vs the fp32 reference (verified: 0/8192 flips,
52x margin at the tightest token). ||e||^2/2 rides as 3 fp16 const rows folded
into the same accumulation (lhsT = -1).
"""
import sys
sys.path.insert(0, "/opt/trn_rl_repo")
from contextlib import ExitStack

import numpy as np

import concourse.bacc as bacc
import concourse.bass as bass
import concourse.mybir as mybir
import concourse.tile as tile
from concourse.bass_utils import run_bass_kernel_spmd
from concourse.masks import make_identity

B, T, D, V = 32, 256, 1280, 8192
NCORES = 8
BPC = B // NCORES           # batches per core
TPC = BPC * T               # tokens per core
NT = TPC // 128             # token tiles per core
KP = D // 128               # contraction part-tiles
CH = 512                    # codes per chunk (PSUM bank)
NCH = V // CH               # chunks
EPS = 1e-5
N_B = T * D                 # layernorm normalization count per batch
IDX_SHIFT = 16384.0         # widx stored as idx - IDX_SHIFT (tie-break => min)

F32 = mybir.dt.float32
F16 = mybir.dt.float16
U32 = mybir.dt.uint32
ALU = mybir.AluOpType
ACTF = mybir.ActivationFunctionType
AX = mybir.AxisListType


def build_nc():
    nc = bacc.Bacc("TRN2", target_bir_lowering=False)

    x_sh = nc.dram_tensor("x_sh", [NT, 128, D], F32, kind="ExternalInput")
    lnw = nc.dram_tensor("lnw", [2, 128, D], F32, kind="ExternalInput")
    lnb = nc.dram_tensor("lnb", [2, 128, D], F32, kind="ExternalInput")
    cbh = nc.dram_tensor("cbh", [KP, 128, V], F16, kind="ExternalInput")
    cbl = nc.dram_tensor("cbl", [KP, 128, V], F16, kind="ExternalInput")
    cbs = nc.dram_tensor("cbs", [3, V], F16, kind="ExternalInput")
    cbg = nc.dram_tensor("cbg", [V, D], F32, kind="ExternalInput")

    out_sh = nc.dram_tensor("out_sh", [NT, 128, D], F32, kind="ExternalOutput")
    loss_p = nc.dram_tensor("loss_p", [128, NT], F32, kind="ExternalOutput")
    idx_o = nc.dram_tensor("idx_o", [128, NT], U32, kind="ExternalOutput")

    f_dram = nc.dram_tensor("f_scr", [NT, 128, D], F32, kind="Internal")

    with tile.TileContext(nc) as tc, ExitStack() as ctx:
        const = ctx.enter_context(tc.tile_pool(name="const", bufs=1))
        big = ctx.enter_context(tc.tile_pool(name="big", bufs=8))
        fpool = ctx.enter_context(tc.tile_pool(name="fpool", bufs=3))
        qpool = ctx.enter_context(tc.tile_pool(name="qpool", bufs=1))
        cbp = ctx.enter_context(tc.tile_pool(name="cbp", bufs=2))
        stg = ctx.enter_context(tc.tile_pool(name="stg", bufs=2))
        wp = ctx.enter_context(tc.tile_pool(name="wp", bufs=1))
        sm = ctx.enter_context(tc.tile_pool(name="sm", bufs=2))
        psA = ctx.enter_context(tc.tile_pool(name="psA", bufs=4, space="PSUM"))
        psT = ctx.enter_context(tc.tile_pool(name="psT", bufs=2, space="PSUM"))
        psS = ctx.enter_context(tc.tile_pool(name="psS", bufs=1, space="PSUM"))

        # ---- constants
        ident = const.tile([128, 128], F32, tag="ident")
        make_identity(nc, ident[:])
        ones128 = const.tile([128, 1], F32, tag="ones128")
        nc.vector.memset(ones128[:], 1.0)
        ones1 = const.tile([1, 128], F32, tag="ones1")
        nc.vector.memset(ones1[:], 1.0)
        neg1 = const.tile([3, 128], F16, tag="neg1")
        nc.vector.memset(neg1[:], -1.0)
        lnw_t = [const.tile([128, D], F32, tag=f"lnw{p}", name=f"lnw{p}") for p in range(2)]
        lnb_t = [const.tile([128, D], F32, tag=f"lnb{p}", name=f"lnb{p}") for p in range(2)]
        for p in range(2):
            nc.sync.dma_start(lnw_t[p][:], lnw[p])
            nc.sync.dma_start(lnb_t[p][:], lnb[p])

        stats16 = const.tile([128, 16], F32, tag="stats16")
        ab_sb = const.tile([128, 8], F32, tag="ab_sb")
        ls_sb = const.tile([128, NT], F32, tag="ls_sb")
        idx_sb = const.tile([128, NT], U32, tag="idx_sb")

        # persistent transposed-q operand tiles
        qh = [qpool.tile([128, TPC], F16, tag=f"qh{k}", name=f"qh{k}") for k in range(KP)]
        q64 = [qpool.tile([128, TPC], F16, tag=f"q64{k}", name=f"q64{k}") for k in range(KP)]
        ql = [qpool.tile([128, TPC], F16, tag=f"ql{k}", name=f"ql{k}") for k in range(KP)]

        wval = [wp.tile([128, NCH], F32, tag=f"wv_{t}", name=f"wv{t}") for t in range(NT)]
        widx = [wp.tile([128, NCH], F32, tag=f"wi_{t}", name=f"wi{t}") for t in range(NT)]

        # ---- phase A: stream x once for per-tile stats (x is re-streamed in
        # phase B; holding all 8 tiles would exceed the shared pool)
        for t in range(NT):
            x_t = big.tile([128, D], F32, tag="big")
            nc.sync.dma_start(x_t[:], x_sh[t])
            nc.vector.tensor_reduce(
                out=stats16[:, t : t + 1], in_=x_t[:], axis=AX.X, op=ALU.add
            )
            scr = big.tile([128, D], F32, tag="big")
            nc.scalar.activation(
                out=scr[:], in_=x_t[:], func=ACTF.Square,
                accum_out=stats16[:, 8 + t : 9 + t],
            )

        # ---- stats -> per-batch alpha/beta, broadcast to all partitions
        ps_st = psS.tile([1, 16], F32, tag="st", space="PSUM")
        nc.tensor.matmul(ps_st[:], ones128[:], stats16[:], start=True, stop=True)
        s16 = sm.tile([1, 16], F32, tag="s16")
        nc.scalar.copy(s16[:], ps_st[:])

        sum4 = sm.tile([1, 4], F32, tag="sum4")
        sq4 = sm.tile([1, 4], F32, tag="sq4")
        nc.vector.tensor_reduce(
            out=sum4[:], in_=s16[:, 0:8].rearrange("p (b two) -> p b two", two=2),
            axis=AX.X, op=ALU.add,
        )
        nc.vector.tensor_reduce(
            out=sq4[:], in_=s16[:, 8:16].rearrange("p (b two) -> p b two", two=2),
            axis=AX.X, op=ALU.add,
        )
        mu4 = sm.tile([1, 4], F32, tag="mu4")
        nc.vector.tensor_scalar_mul(mu4[:], sum4[:], 1.0 / N_B)
        e24 = sm.tile([1, 4], F32, tag="e24")
        nc.vector.tensor_scalar_mul(e24[:], sq4[:], 1.0 / N_B)
        var4 = sm.tile([1, 4], F32, tag="var4")
        nc.vector.tensor_tensor(out=var4[:], in0=mu4[:], in1=mu4[:], op=ALU.mult)
        nc.vector.tensor_tensor(out=var4[:], in0=e24[:], in1=var4[:], op=ALU.subtract)
        nc.vector.tensor_scalar_add(var4[:], var4[:], EPS)
        std4 = sm.tile([1, 4], F32, tag="std4")
        nc.scalar.sqrt(std4[:], var4[:])
        ab8 = sm.tile([1, 8], F32, tag="ab8")
        nc.vector.reciprocal(ab8[:, 0:4], std4[:])
        nmu4 = sm.tile([1, 4], F32, tag="nmu4")
        nc.vector.tensor_scalar_mul(nmu4[:], mu4[:], -1.0)
        nc.vector.tensor_tensor(
            out=ab8[:, 4:8], in0=nmu4[:], in1=ab8[:, 0:4], op=ALU.mult
        )
        ps_ab = psS.tile([128, 8], F32, tag="ab", space="PSUM")
        nc.tensor.matmul(ps_ab[:], ones1[:], ab8[:], start=True, stop=True)
        nc.scalar.copy(ab_sb[:], ps_ab[:])

        # ---- phase B: f = (x*alpha + beta)*lnw + lnb ; phase C: transpose+split
        for t in range(NT):
            b, p = t // 2, t % 2
            x_t = big.tile([128, D], F32, tag="big")
            nc.sync.dma_start(x_t[:], x_sh[t])
            u = big.tile([128, D], F32, tag="big")
            nc.scalar.activation(
                out=u[:], in_=x_t[:], func=ACTF.Identity,
                bias=ab_sb[:, 4 + b : 5 + b], scale=ab_sb[:, b : b + 1],
            )
            v = big.tile([128, D], F32, tag="big")
            nc.vector.tensor_tensor(out=v[:], in0=u[:], in1=lnw_t[p][:], op=ALU.mult)
            f_t = fpool.tile([128, D], F32, tag="f")
            nc.vector.tensor_tensor(out=f_t[:], in0=v[:], in1=lnb_t[p][:], op=ALU.add)
            nc.sync.dma_start(f_dram[t], f_t[:])

            cs = slice(128 * t, 128 * (t + 1))
            for k in range(KP):
                tr = psT.tile([128, 128], F32, tag="tr", space="PSUM")
                nc.tensor.transpose(
                    out=tr[:], in_=f_t[:, 128 * k : 128 * (k + 1)], identity=ident[:]
                )
                nc.scalar.copy(qh[k][:, cs], tr[:])
                nc.scalar.mul(q64[k][:, cs], tr[:], 1.0 / 64.0)
                nc.vector.tensor_tensor(
                    out=ql[k][:, cs], in0=tr[:], in1=qh[k][:, cs], op=ALU.subtract
                )

        # ---- phase D: chunked ns matmuls + per-chunk argmax
        for n in range(NCH):
            ncs = slice(CH * n, CH * (n + 1))
            ch_t = []
            cl_t = []
            for k in range(KP):
                c1 = cbp.tile([128, CH], F16, tag=f"ch{k}", name=f"ch{k}_{n}")
                nc.sync.dma_start(c1[:], cbh[k, :, ncs])
                ch_t.append(c1)
                c2 = cbp.tile([128, CH], F16, tag=f"cl{k}", name=f"cl{k}_{n}")
                nc.sync.dma_start(c2[:], cbl[k, :, ncs])
                cl_t.append(c2)
            cs_t = cbp.tile([3, CH], F16, tag="cs")
            nc.sync.dma_start(cs_t[:], cbs[:, ncs])

            for t in range(NT):
                tcs = slice(128 * t, 128 * (t + 1))
                pt = psA.tile([128, CH], F32, tag="mm", space="PSUM")
                for k in range(KP):
                    nc.tensor.matmul(
                        pt[:], qh[k][:, tcs], ch_t[k][:], start=(k == 0), stop=False
                    )
                for k in range(KP):
                    nc.tensor.matmul(pt[:], q64[k][:, tcs], cl_t[k][:],
                                     start=False, stop=False)
                for k in range(KP):
                    nc.tensor.matmul(pt[:], ql[k][:, tcs], ch_t[k][:],
                                     start=False, stop=False)
                nc.tensor.matmul(pt[:], neg1[:], cs_t[:], start=False, stop=True)

                st = stg.tile([128, CH], F32, tag="st")
                nc.scalar.copy(st[:], pt[:])
                mx8 = sm.tile([128, 8], F32, tag="mx8")
                nc.vector.max(out=mx8[:], in_=st[:])
                nc.vector.tensor_copy(wval[t][:, n : n + 1], mx8[:, 0:1])
                mxi = sm.tile([128, 8], U32, tag="mxi")
                nc.vector.max_index(out=mxi[:], in_max=mx8[:], in_values=st[:])
                # widx = (local_idx + n*CH) - IDX_SHIFT, stored fp32 (negative)
                nc.vector.tensor_scalar_add(
                    widx[t][:, n : n + 1], mxi[:, 0:1], float(n * CH) - IDX_SHIFT
                )

        # ---- phase E: final argmax, gather, output, loss
        for t in range(NT):
            fm = sm.tile([128, 8], F32, tag="fm")
            nc.vector.max(out=fm[:], in_=wval[t][:])
            weq = sm.tile([128, NCH], F32, tag="weq")
            nc.vector.tensor_tensor(
                out=weq[:], in0=wval[t][:], in1=fm[:, 0:1].to_broadcast([128, NCH]),
                op=ALU.is_equal,
            )
            wm = sm.tile([128, NCH], F32, tag="wm")
            nc.vector.tensor_tensor(out=wm[:], in0=weq[:], in1=widx[t][:], op=ALU.mult)
            gif = sm.tile([128, 1], F32, tag="gif")
            nc.vector.tensor_reduce(out=gif[:], in_=wm[:], axis=AX.X, op=ALU.min)
            gidx = wp.tile([128, 1], U32, tag=f"gx{t}", name=f"gx{t}")
            nc.vector.tensor_scalar_add(gidx[:], gif[:], IDX_SHIFT)
            nc.vector.tensor_copy(idx_sb[:, t : t + 1], gidx[:])

            e_t = big.tile([128, D], F32, tag="big")
            nc.gpsimd.indirect_dma_start(
                out=e_t[:], out_offset=None, in_=cbg[:],
                in_offset=bass.IndirectOffsetOnAxis(ap=gidx[:, :1], axis=0),
            )
            f_rl = big.tile([128, D], F32, tag="big")
            nc.sync.dma_start(f_rl[:], f_dram[t])
            d_t = big.tile([128, D], F32, tag="big")
            nc.vector.tensor_tensor(out=d_t[:], in0=e_t[:], in1=f_rl[:], op=ALU.subtract)
            o_t = big.tile([128, D], F32, tag="big")
            nc.vector.tensor_tensor(out=o_t[:], in0=f_rl[:], in1=d_t[:], op=ALU.add)
            nc.sync.dma_start(out_sh[t], o_t[:])
            scrE = big.tile([128, D], F32, tag="big")
            nc.scalar.activation(
                out=scrE[:], in_=d_t[:], func=ACTF.Square,
                accum_out=ls_sb[:, t : t + 1],
            )

        nc.sync.dma_start(loss_p[:], ls_sb[:])
        nc.sync.dma_start(idx_o[:], idx_sb[:])

    nc.compile()
    return nc


def prep_shared(codebook):
    cb = np.asarray(codebook, np.float32)
    eh = cb.astype(np.float16)
    el64 = ((cb - eh.astype(np.float32)) * 64.0).astype(np.float16)
    cbh = np.ascontiguousarray(
        eh.T.reshape(KP, 128, V)
    )
    cbl = np.ascontiguousarray(el64.T.reshape(KP, 128, V))
    sq = (cb.astype(np.float64) ** 2).sum(1).astype(np.float32) * 0.5
    s1 = sq.astype(np.float16)
    s2 = (sq - s1.astype(np.float32)).astype(np.float16)
    s3 = (sq - s1.astype(np.float32) - s2.astype(np.float32)).astype(np.float16)
    cbs = np.stack([s1, s2, s3])
    return cbh, cbl, cbs, cb


_cached_nc = None


def kernel(x, ln_weight, ln_bias, codebook):
    global _cached_nc
    x = np.asarray(x, np.float32)
    lnw = np.ascontiguousarray(np.asarray(ln_weight, np.float32).reshape(2, 128, D))
    lnb = np.ascontiguousarray(np.asarray(ln_bias, np.float32).reshape(2, 128, D))
    cbh, cbl, cbs, cbg = prep_shared(codebook)

    if _cached_nc is None:
        _cached_nc = build_nc()
    nc = _cached_nc

    in_maps = []
    for c in range(NCORES):
        xs = np.ascontiguousarray(
            x[BPC * c : BPC * (c + 1)].reshape(NT, 128, D)
        )
        in_maps.append(
            {"x_sh": xs, "lnw": lnw, "lnb": lnb, "cbh": cbh, "cbl": cbl,
             "cbs": cbs, "cbg": cbg}
        )

    res = run_bass_kernel_spmd(nc, in_maps, core_ids=list(range(NCORES)))

    out = np.empty((B, T, D), np.float32)
    total = 0.0
    for c, r in enumerate(res.results):
        out[BPC * c : BPC * (c + 1)] = r["out_sh"].reshape(BPC, T, D)
        total += r["loss_p"].astype(np.float64).sum()
    loss = np.float32(1.25 * total / (B * T * D))
    return out, loss
